# revision 13
# baseline (speedup 1.0000x reference)
"""Trainium2 Bass kernel for a dense transformer block (GQA attention + GeGLU MLP).

Sharding: 8 cores = 2 batches x 4 token-slices of 512 tokens. Each core runs the
full block for its 512 query tokens; the (small) KV projection over the whole
batch is recomputed per core, so there is no cross-core communication.

Attention runs in bf16. The MLP matmuls run in fp8(e4m3) DoubleRow mode with
same-scale hi/lo decompositions to keep near-bf16 accuracy:
  - gate/up: weights split w = w_hi + w_lo; activations h = h_hi + h_lo.
    Per k-tile one DR instruction computes (w_hi + w_lo)@h_hi, and one DR
    instruction per k-tile PAIR adds w_hi@h_lo for both tiles (w_lo@h_lo is
    dropped: ~0.1% of the product). 3 instructions per 2 k-tiles vs 4 for bf16.
  - linear: weights hi/lo, activations single fp8 stored twice so one DR
    instruction per f-tile computes (wl_hi + wl_lo)@acts.

Layout convention on device: activations live transposed, [feature, token],
so every matmul contracts over the partition dim with natural operands.
"""

import sys
import threading

import numpy as np

sys.path.insert(0, "/opt/trn_rl_repo")

import ml_dtypes

import concourse.bass as bass
import concourse.tile as tile
from concourse import bacc
from concourse import mybir
from concourse.masks import make_identity

P = 128
F32 = mybir.dt.float32
BF16 = mybir.dt.bfloat16
FP8 = mybir.dt.float8e4
NPBF16 = ml_dtypes.bfloat16
NPFP8 = ml_dtypes.float8_e4m3
DR = mybir.MatmulPerfMode.DoubleRow

S_H2 = 32.0     # fp8 scale of the normalized mid residual (|h2|max ~ 5.5)
S_ACTS = 8.0    # fp8 scale of the geglu activations (|acts|max ~ 16)

FULL_CFG = dict(
    DT=16,    # d tiles (D = DT*128)
    ST=16,    # s tiles (S = ST*128) keys per batch
    NH=8,     # query heads
    HC=2,     # head-dim chunks (head dim = HC*128)
    QC=4,     # query-token chunks (QT = QC*128)
    FT=128,   # f tiles (F = FT*128)
    JPT=32,   # f tiles per portion
    NDB=4,    # d batches in linear sweep
    DBW=4,    # d chunks per linear batch
    EPS=1e-6,
)


def _cfg_dims(cfg):
    D = cfg["DT"] * P
    S = cfg["ST"] * P
    QT = cfg["QC"] * P
    F = cfg["FT"] * P
    HD = cfg["HC"] * P
    return D, S, QT, F, HD


def build_nc(wscales, cfg=FULL_CFG):
    DT, ST, NH, HC = cfg["DT"], cfg["ST"], cfg["NH"], cfg["HC"]
    QC, FT, JPT, NDB, DBW = cfg["QC"], cfg["FT"], cfg["JPT"], cfg["NDB"], cfg["DBW"]
    D, S, QT, F, HD = _cfg_dims(cfg)
    NPORT = FT // JPT
    assert NDB * DBW == DT
    EPS = cfg["EPS"]
    SC = S // 512          # 512-token chunks of the full batch
    CW = 512               # chunk width

    s_w0, s_w1, s_wl = wscales["s_w0"], wscales["s_w1"], wscales["s_wl"]
    gel_scale = 1.0 / (S_H2 * s_w0)
    lam_up = S_ACTS / (S_H2 * s_w1)
    lin_scale = 1.0 / (S_ACTS * s_wl)

    nc = bacc.Bacc(None)

    # ---- per-core inputs (host-prepared) ----
    xT_d = nc.dram_tensor("xT_bf", [P, DT, S], BF16, kind="ExternalInput")
    xTq_d = nc.dram_tensor("xTq_f32", [P, DT, QT], F32, kind="ExternalInput")
    xTqb_d = nc.dram_tensor("xTq_bf", [P, DT, QT], BF16, kind="ExternalInput")
    cosk_d = nc.dram_tensor("cosk", [P, S], F32, kind="ExternalInput")
    sink_d = nc.dram_tensor("sink", [P, S], F32, kind="ExternalInput")
    cosq_d = nc.dram_tensor("cosq", [P, QT], F32, kind="ExternalInput")
    sinq_d = nc.dram_tensor("sinq", [P, QT], F32, kind="ExternalInput")
    mask_d = nc.dram_tensor("maskT", [P, ST, QT], BF16, kind="ExternalInput")
    qw_d = nc.dram_tensor("qw_t", [NH * HC, P, DT, P], BF16, kind="ExternalInput")
    kw_d = nc.dram_tensor("kw_t", [HC, P, DT, P], BF16, kind="ExternalInput")
    vw_d = nc.dram_tensor("vw_t", [P, DT, HD], BF16, kind="ExternalInput")
    avw_d = nc.dram_tensor("avw_t", [DT, P, NH * HC, P], BF16, kind="ExternalInput")
    # fp8 hi/lo MLP weights: w01hl[ft, di, k, g, hl, fi], wlhl[blk, fi, j4, hl, dc, di]
    w01_d = nc.dram_tensor("w01_t", [FT, P, DT, 2, 2, P], FP8,
                           kind="ExternalInput")
    wl_d = nc.dram_tensor("wl_t", [NPORT * NDB * (JPT // 4), P, 4, 2, DBW, P],
                          FP8, kind="ExternalInput")
    out_d = nc.dram_tensor("outT", [P, DT, QT], F32, kind="ExternalOutput")

    MULT = mybir.AluOpType.mult
    ADD = mybir.AluOpType.add
    SUB = mybir.AluOpType.subtract
    AF = mybir.ActivationFunctionType

    with tile.TileContext(nc) as tc:
        with (
            tc.tile_pool(name="glob", bufs=1) as glob,
            tc.tile_pool(name="misc", bufs=2) as misc,
            tc.tile_pool(name="dramp", bufs=1, space="DRAM") as dramp,
        ):
            ident = glob.tile([P, P], BF16, tag="ident")
            make_identity(nc, ident)
            id32 = glob.tile([P, P], F32, tag="id32")
            make_identity(nc, id32)
            ones_col = glob.tile([P, 1], BF16, tag="ones_col")
            nc.vector.memset(ones_col, 1.0)
            eps_b = glob.tile([P, 1], F32, tag="eps_b")
            nc.vector.memset(eps_b, EPS)
            eps2_b = glob.tile([P, 1], F32, tag="eps2_b")
            nc.vector.memset(eps2_b, EPS / (S_H2 * S_H2))
            zero_b = glob.tile([P, 1], F32, tag="zero_b")
            nc.vector.memset(zero_b, 0.0)

            # PE warmup: keep TensorE busy while input DMAs land so the
            # HAM clock gate opens before real matmuls arrive.
            with tc.tile_pool(name="warm", bufs=1, space="PSUM") as warmp:
                wps = warmp.tile([P, P], F32, tag="warm")
                for wi in range(40):
                    nc.tensor.matmul(wps, ident[:], ident[:],
                                     start=(wi == 0), stop=(wi == 39))

            # cross-phase tensors
            kT = glob.tile([P, HC, S], BF16, tag="kT")
            v_sb = glob.tile([P, ST, HD + 1], BF16, tag="v_sb")
            qT = glob.tile([P, NH * HC, QT], BF16, tag="qT")
            xTq = glob.tile([P, DT, QT], F32, tag="bigf32")   # later reused by h2T
            x_newT = glob.tile([P, DT, QT], F32, tag="x_newT")

            def rstd_chunk(sq, ps_pool, ps_tag, nat, col0, nq, sc, bias):
                """sq: [P, DT, nq*128] bf16; writes rstd into nat[:, col0:col0+nq]."""
                for qq in range(nq):
                    var_ps = ps_pool.tile([P, 1], F32, tag=ps_tag, bufs=2, name="var_ps")
                    for dt_i in range(DT):
                        nc.tensor.matmul(
                            var_ps, sq[:, dt_i, qq * P:(qq + 1) * P], ones_col,
                            start=(dt_i == 0), stop=(dt_i == DT - 1))
                    std = misc.tile([P, 1], F32, tag="std")
                    nc.scalar.activation(std, var_ps, AF.Sqrt,
                                         bias=bias[:], scale=sc)
                    nc.vector.reciprocal(nat[:, col0 + qq:col0 + qq + 1], std)

            def rstd_broadcast(nat, n_tiles, ps_pool, ps_tag2, bc_pool, bc_tag):
                """nat: [P, n_tiles] f32 -> [P, n_tiles*128] broadcast tile."""
                tps = ps_pool.tile([P, P], F32, tag=ps_tag2, bufs=2, name="tps")
                nc.tensor.transpose(tps[:n_tiles, :], nat[:], id32)
                row_sb = misc.tile([n_tiles, P], F32, tag=f"row{n_tiles}", bufs=1)
                nc.vector.tensor_copy(row_sb[:], tps[:n_tiles, :])
                scr = dramp.tile([n_tiles, P], F32, tag=f"scr{bc_tag}")
                nc.sync.dma_start(scr[:], row_sb[:])
                bc = bc_pool.tile([P, n_tiles * P], F32, tag=bc_tag, bufs=1, name="bc")
                src_ap = bass.AP(tensor=scr.tensor, offset=scr.offset,
                                 ap=[[0, P], [1, n_tiles * P]])
                nc.gpsimd.dma_start(bc[:], src_ap)
                return bc

            # =========== Phase P: norms, projections, RoPE ===========
            with (
                tc.tile_pool(name="pp", bufs=1) as pp,
                tc.tile_pool(name="ppr", bufs=2) as ppr,
                tc.tile_pool(name="wtile", bufs=2) as wtile,
                tc.tile_pool(name="psP", bufs=1, space="PSUM") as psP,
            ):
                # ---- N1q: rmsnorm of this core's q slice ----
                xTqb = pp.tile([P, DT, QT], BF16, tag="big16", bufs=2,
                               name="xTqb")
                sqq = pp.tile([P, DT, QT], BF16, tag="sq", bufs=1, name="sqq")
                natq = misc.tile([P, QC], F32, tag="natq")
                for qq in range(QC):
                    qsl = slice(qq * P, (qq + 1) * P)
                    nc.sync.dma_start(xTqb[:, :, qsl], xTqb_d[:, :, qsl])
                    nc.vector.tensor_tensor(sqq[:, :, qsl], xTqb[:, :, qsl],
                                            xTqb[:, :, qsl], MULT)
                    rstd_chunk(sqq[:, :, qsl], psP, "var", natq, qq, 1,
                               1.0 / D, eps_b)
                nc.sync.dma_start(xTq[:], xTq_d[:])
                bcq = rstd_broadcast(natq, QC, psP, "tps", ppr, "bcq")
                normTq = pp.tile([P, DT, QT], BF16, tag="nq16", bufs=1,
                                 name="normTq")
                for dt_i in range(DT):
                    nc.vector.tensor_tensor(normTq[:, dt_i, :], xTq[:, dt_i, :],
                                            bcq[:], MULT)

                cosq = pp.tile([P, QT], F32, tag="cosq")
                nc.sync.dma_start(cosq[:], cosq_d[:])
                sinq = pp.tile([P, QT], F32, tag="sinq")
                nc.sync.dma_start(sinq[:], sinq_d[:])

                def rope_pair(x1_ps, x2_ps, cos_sl, sin_sl, out1, out2, w):
                    t1 = ppr.tile([P, w], F32, tag="ropeA", bufs=1, name="t1")
                    t2 = ppr.tile([P, w], F32, tag="ropeB", bufs=1, name="t2")
                    nc.vector.tensor_tensor(t1[:], x1_ps[:], cos_sl, MULT)
                    nc.vector.tensor_tensor(t2[:], x2_ps[:], sin_sl, MULT)
                    nc.vector.tensor_tensor(out1, t1[:], t2[:], SUB)
                    t3 = ppr.tile([P, w], F32, tag="ropeA", bufs=1, name="t3")
                    t4 = ppr.tile([P, w], F32, tag="ropeB", bufs=1, name="t4")
                    nc.vector.tensor_tensor(t3[:], x2_ps[:], cos_sl, MULT)
                    nc.vector.tensor_tensor(t4[:], x1_ps[:], sin_sl, MULT)
                    nc.vector.tensor_tensor(out2, t3[:], t4[:], ADD)

                def q_head(n):
                    qps = []
                    for hc in range(HC):
                        qw_sb = wtile.tile([P, DT, P], BF16, tag="wt",
                                           name="qw_sb")
                        nc.sync.dma_start(qw_sb[:], qw_d[n * HC + hc])
                        qp = psP.tile([P, QT], F32, tag="pj", bufs=4, name="qp")
                        for dt_i in range(DT):
                            nc.tensor.matmul(qp, qw_sb[:, dt_i, :],
                                             normTq[:, dt_i, :],
                                             start=(dt_i == 0),
                                             stop=(dt_i == DT - 1))
                        qps.append(qp)
                    rope_pair(qps[0], qps[1], cosq[:], sinq[:],
                              qT[:, n * HC, :], qT[:, n * HC + 1, :], QT)

                q_head(0)

                # ---- full-batch stats pass: rstd for every s-chunk ----
                bc_list = []
                for c in range(SC):
                    csl = slice(c * CW, (c + 1) * CW)
                    xTa = pp.tile([P, DT, CW], BF16, tag="big16", bufs=2,
                                  name="xTa")
                    nc.sync.dma_start(xTa[:], xT_d[:, :, csl])
                    sqc = pp.tile([P, DT, CW], BF16, tag="sq", bufs=1,
                                  name="sqc")
                    nc.vector.tensor_tensor(sqc[:], xTa[:], xTa[:], MULT)
                    natc = misc.tile([P, CW // P], F32, tag="natc")
                    rstd_chunk(sqc, psP, "var", natc, 0, CW // P, 1.0 / D,
                               eps_b)
                    bcc = rstd_broadcast(natc, CW // P, psP, "var", ppr,
                                         f"bcc{c}")
                    bc_list.append(bcc)

                for n in range(1, NH):
                    q_head(n)
                nc.sync.dma_start(xTq[:], xTq_d[:])

                # ---- norm + K/V projections (re-load x chunks) ----
                kw_sbs = []
                for hc in range(HC):
                    kw_sb = wtile.tile([P, DT, P], BF16, tag="wt",
                                       name="kw_sb")
                    nc.sync.dma_start(kw_sb[:], kw_d[hc])
                    kw_sbs.append(kw_sb)
                vw_sb = wtile.tile([P, DT, HD], BF16, tag="vwt", bufs=1,
                                   name="vw_sb")
                nc.sync.dma_start(vw_sb[:], vw_d[:])
                nc.vector.memset(v_sb[:], 1.0)

                for c in range(SC):
                    csl = slice(c * CW, (c + 1) * CW)
                    xTc = pp.tile([P, DT, CW], BF16, tag="big16", bufs=2,
                                  name="xTc")
                    nc.sync.dma_start(xTc[:], xT_d[:, :, csl])
                    for dt_i in range(DT):
                        nc.vector.tensor_tensor(xTc[:, dt_i, :], xTc[:, dt_i, :],
                                                bc_list[c][:], MULT)
                    # K^T for this chunk
                    cos_c = ppr.tile([P, CW], F32, tag="cos_c", bufs=1, name="cos_c")
                    nc.sync.dma_start(cos_c[:], cosk_d[:, csl])
                    sin_c = ppr.tile([P, CW], F32, tag="sin_c", bufs=1, name="sin_c")
                    nc.sync.dma_start(sin_c[:], sink_d[:, csl])
                    kps = []
                    for hc in range(HC):
                        kp = psP.tile([P, CW], F32, tag="pj", bufs=4, name="kp")
                        for dt_i in range(DT):
                            nc.tensor.matmul(kp, kw_sbs[hc][:, dt_i, :],
                                             xTc[:, dt_i, :],
                                             start=(dt_i == 0),
                                             stop=(dt_i == DT - 1))
                        kps.append(kp)
                    rope_pair(kps[0], kps[1], cos_c[:], sin_c[:],
                              kT[:, 0, csl], kT[:, 1, csl], CW)
                    # V for this chunk
                    for ss in range(CW // P):
                        st_i = c * (CW // P) + ss
                        vp = psP.tile([P, HD], F32, tag="pj", bufs=4, name="vp")
                        for dt_i in range(DT):
                            nc.tensor.matmul(
                                vp, xTc[:, dt_i, ss * P:(ss + 1) * P],
                                vw_sb[:, dt_i, :],
                                start=(dt_i == 0), stop=(dt_i == DT - 1))
                        nc.scalar.copy(v_sb[:, st_i, :HD], vp[:])

            # =========== Phase A: attention, out-proj, N2 ===========
            with (
                tc.tile_pool(name="pa", bufs=1) as pa,
                tc.tile_pool(name="expp", bufs=2) as expp,
                tc.tile_pool(name="avwp", bufs=3) as avwp,
            ):
                mask_sb = pa.tile([P, ST, QT], BF16, tag="m16", name="mask_sb")
                nc.sync.dma_start(mask_sb[:], mask_d[:])
                enc_sb = pa.tile([P, QC, NH, HD], BF16, tag="enc_sb")
                encT = pa.tile([P, NH * HC, QT], BF16, tag="encT")
                exp_tiles = [None] * NH

                with tc.tile_pool(name="psA", bufs=1, space="PSUM") as psA:

                    def emit_logits(n):
                        expT = expp.tile([P, ST, QT], BF16, tag="expT",
                                         name="expT")
                        exp_tiles[n] = expT
                        for st_i in range(ST):
                            lp = psA.tile([P, QT], F32, tag="lps", bufs=4,
                                          name="lp")
                            for hc in range(HC):
                                nc.tensor.matmul(
                                    lp, kT[:, hc, st_i * P:(st_i + 1) * P],
                                    qT[:, n * HC + hc, :],
                                    start=(hc == 0), stop=(hc == HC - 1))
                            nc.scalar.activation(expT[:, st_i, :], lp, AF.Exp,
                                                 bias=zero_b[:])
                            nc.vector.tensor_tensor(expT[:, st_i, :],
                                                    expT[:, st_i, :],
                                                    mask_sb[:, st_i, :], MULT)

                    def emit_av(n):
                        expT = exp_tiles[n]
                        for qc in range(QC):
                            ap_ = psA.tile([P, HD + 1], F32, tag="avps",
                                           bufs=2, name="ap_")
                            for st_i in range(ST):
                                nc.tensor.matmul(
                                    ap_, expT[:, st_i, qc * P:(qc + 1) * P],
                                    v_sb[:, st_i, :],
                                    start=(st_i == 0), stop=(st_i == ST - 1))
                            ssum = misc.tile([P, 1], F32, tag="ssum")
                            nc.vector.tensor_copy(ssum[:], ap_[:, HD:HD + 1])
                            rec = misc.tile([P, 1], F32, tag="rec")
                            nc.vector.reciprocal(rec[:], ssum[:])
                            nc.vector.tensor_scalar_mul(enc_sb[:, qc, n, :],
                                                        ap_[:, :HD], rec[:])

                    for n in range(NH):
                        emit_logits(n)
                        if n > 0:
                            emit_av(n - 1)
                    emit_av(NH - 1)

                    # transpose enc -> encT [nh*hd, qt]
                    for qc in range(QC):
                        for nt in range(NH * HC):
                            n, hs = nt // HC, nt % HC
                            tp = psA.tile([P, P], BF16, tag="lps", bufs=4,
                                          name="tp")
                            nc.tensor.transpose(
                                tp, enc_sb[:, qc, n, hs * P:(hs + 1) * P],
                                ident)
                            nc.vector.tensor_copy(
                                encT[:, nt, qc * P:(qc + 1) * P], tp[:])

                with tc.tile_pool(name="psA2", bufs=1, space="PSUM") as psA2:
                    # out-projection (transposed) + residual; N2 variance
                    # matmuls accumulate inside the loop.
                    sq2 = pa.tile([P, DT, QT], BF16, tag="m16", name="sq2")
                    var2 = [
                        psA2.tile([P, 1], F32, tag=f"n2v{i}", bufs=1,
                                  name=f"n2v{i}")
                        for i in range(QC)
                    ]
                    for dc in range(DT):
                        avw_sb = avwp.tile([P, NH * HC, P], BF16, tag="avw",
                                           name="avw_sb")
                        nc.scalar.dma_start(avw_sb[:], avw_d[dc])
                        op_ = psA2.tile([P, QT], F32, tag="ops", bufs=2,
                                        name="op_")
                        for nt in range(NH * HC):
                            nc.tensor.matmul(op_, avw_sb[:, nt, :],
                                             encT[:, nt, :],
                                             start=(nt == 0),
                                             stop=(nt == NH * HC - 1))
                        nc.vector.tensor_tensor(x_newT[:, dc, :], op_[:],
                                                xTq[:, dc, :], ADD)
                        nc.vector.tensor_tensor(sq2[:, dc, :],
                                                x_newT[:, dc, :],
                                                x_newT[:, dc, :], MULT)
                        for qq in range(QC):
                            nc.tensor.matmul(
                                var2[qq], sq2[:, dc, qq * P:(qq + 1) * P],
                                ones_col, start=(dc == 0), stop=(dc == DT - 1))

                    # ---- N2 tail: bc2 carries the fp8 scale S_H2 ----
                    nat2 = misc.tile([P, QC], F32, tag="nat2")
                    for qq in range(QC):
                        std2 = misc.tile([P, 1], F32, tag="std")
                        nc.scalar.activation(std2, var2[qq], AF.Sqrt,
                                             bias=eps2_b[:],
                                             scale=1.0 / (D * S_H2 * S_H2))
                        nc.vector.reciprocal(nat2[:, qq:qq + 1], std2)
                    bc2 = rstd_broadcast(nat2, QC, psA2, "ops", pa, "bc2")
                    h2T = glob.tile([P, DT, QT], BF16, tag="bigf32", name="h2T")
                    for dt_i in range(DT):
                        nc.vector.tensor_tensor(h2T[:, dt_i, :],
                                                x_newT[:, dt_i, :], bc2[:],
                                                MULT)

            # =========== Phase M: GeGLU MLP (fp8 hi/lo) + residual ===========
            with (
                tc.tile_pool(name="actsp", bufs=1) as actsp,
                tc.tile_pool(name="w01p", bufs=2) as w01p,
                tc.tile_pool(name="wlp", bufs=3) as wlp,
                tc.tile_pool(name="gelp", bufs=3) as gelp,
                tc.tile_pool(name="upscp", bufs=3) as upscp,
                tc.tile_pool(name="linsp", bufs=3) as linsp,
                tc.tile_pool(name="psM", bufs=1, space="PSUM") as psM,
            ):
                # hi/lo split of h2 (planes: hi, hi, lo) — h2T = S_H2 * normed
                h2q = actsp.tile([P, DT, 3, QT], FP8, tag="h2q", name="h2q")
                for dt_i in range(DT):
                    nc.vector.tensor_copy(h2q[:, dt_i, 0, :], h2T[:, dt_i, :])
                    nc.scalar.copy(h2q[:, dt_i, 1, :], h2q[:, dt_i, 0, :])
                    nc.vector.tensor_tensor(h2q[:, dt_i, 2, :], h2T[:, dt_i, :],
                                            h2q[:, dt_i, 0, :], SUB)

                def geglu_gemm(gout, w01_sb, g):
                    # (w_hi + w_lo) @ h_hi for each k-tile, then w_hi @ h_lo
                    # for k-tile pairs; all accumulate into gout.
                    for k in range(DT):
                        nc.tensor.matmul(gout, w01_sb[:, k, g, :, :],
                                         h2q[:, k, 0:2, :],
                                         start=(k == 0), stop=False,
                                         perf_mode=DR)
                    for kp in range(DT // 2):
                        nc.tensor.matmul(gout,
                                         w01_sb[:, 2 * kp:2 * kp + 2, g, 0, :],
                                         h2q[:, 2 * kp:2 * kp + 2, 2, :],
                                         start=False, stop=(kp == DT // 2 - 1),
                                         perf_mode=DR)

                for p in range(NPORT):
                    actsT = actsp.tile([P, JPT, 2, QT], FP8, tag="acts",
                                       name="actsT")
                    for j in range(JPT):
                        ft = p * JPT + j
                        w01_sb = w01p.tile([P, DT, 2, 2, P], FP8, tag="w01",
                                           name="w01_sb")
                        nc.sync.dma_start(w01_sb[:], w01_d[ft])
                        gp = psM.tile([P, QT], F32, tag="gps", bufs=2, name="gp")
                        geglu_gemm(gp, w01_sb, 0)
                        gel = gelp.tile([P, QT], BF16, tag="gel", name="gel")
                        nc.scalar.activation(gel[:], gp, AF.Gelu_apprx_tanh,
                                             bias=zero_b[:], scale=gel_scale)
                        up = psM.tile([P, QT], F32, tag="ups", bufs=2, name="up")
                        geglu_gemm(up, w01_sb, 1)
                        up_sc = upscp.tile([P, QT], BF16, tag="upsc",
                                           name="up_sc")
                        nc.scalar.activation(up_sc[:], up, AF.Copy,
                                             scale=lam_up)
                        nc.vector.tensor_tensor(actsT[:, j, 0, :], up_sc[:],
                                                gel[:], MULT)
                        nc.vector.tensor_copy(actsT[:, j, 1, :],
                                              actsT[:, j, 0, :])
                    for db in range(NDB):
                        lps = [
                            psM.tile([P, QT], F32, tag=f"lin{i}", bufs=1,
                                     name=f"lin{i}")
                            for i in range(DBW)
                        ]
                        for jb in range(JPT // 4):
                            wl_sb = wlp.tile([P, 4, 2, DBW, P], FP8, tag="wl",
                                             name="wl_sb")
                            nc.scalar.dma_start(
                                wl_sb[:], wl_d[(p * NDB + db) * (JPT // 4) + jb])
                            for j4 in range(4):
                                j = jb * 4 + j4
                                for dc4 in range(DBW):
                                    nc.tensor.matmul(lps[dc4],
                                                     wl_sb[:, j4, :, dc4, :],
                                                     actsT[:, j, :, :],
                                                     start=(j == 0),
                                                     stop=(j == JPT - 1),
                                                     perf_mode=DR)
                        for dc4 in range(DBW):
                            d_i = db * DBW + dc4
                            lin_sc = linsp.tile([P, QT], F32, tag="linsc",
                                                name="lin_sc")
                            nc.scalar.activation(lin_sc[:], lps[dc4], AF.Copy,
                                                 scale=lin_scale)
                            nc.vector.tensor_tensor(x_newT[:, d_i, :],
                                                    lin_sc[:],
                                                    x_newT[:, d_i, :], ADD)
                        if p == NPORT - 1:
                            nc.sync.dma_start(
                                out_d[:, db * DBW:(db + 1) * DBW, :],
                                x_newT[:, db * DBW:(db + 1) * DBW, :])

    nc.compile()
    return nc


# ---------------------------------------------------------------------------
# Host-side packing
# ---------------------------------------------------------------------------

def weight_scales(inputs):
    gating_w = np.asarray(inputs["gating_w"], dtype=np.float32)
    pf = np.asarray(inputs["pre_ffw_scale"], dtype=np.float32)
    w0 = gating_w[0] * (1.0 + pf)[:, None]
    w1 = gating_w[1] * (1.0 + pf)[:, None]
    return dict(
        s_w0=240.0 / np.abs(w0).max(),
        s_w1=240.0 / np.abs(w1).max(),
        s_wl=240.0 / np.abs(inputs["linear_w"]).max(),
    )


def pack_inputs(inputs, wscales, cfg=FULL_CFG, n_cores=8):
    D, S, QT, F, HD = _cfg_dims(cfg)
    DT, ST, NH, HC = cfg["DT"], cfg["ST"], cfg["NH"], cfg["HC"]
    QC, FT, JPT, NDB, DBW = cfg["QC"], cfg["FT"], cfg["JPT"], cfg["NDB"], cfg["DBW"]
    NPORT = FT // JPT

    x = np.asarray(inputs["x"], dtype=np.float32)            # [B, T, D]
    positions = np.asarray(inputs["positions"])              # [B, T]
    attn_mask = np.asarray(inputs["attn_mask"])              # [B, 1, T, T]
    q_w = np.asarray(inputs["q_w"], dtype=np.float32)        # [NH, D, HD]
    kv_w = np.asarray(inputs["kv_w"], dtype=np.float32)      # [2, 1, D, HD]
    attn_vec_w = np.asarray(inputs["attn_vec_w"], dtype=np.float32)  # [NH, HD, D]
    gating_w = np.asarray(inputs["gating_w"], dtype=np.float32)      # [2, D, F]
    linear_w = np.asarray(inputs["linear_w"], dtype=np.float32)      # [F, D]
    pa = np.asarray(inputs["pre_attn_scale"], dtype=np.float32)
    pf = np.asarray(inputs["pre_ffw_scale"], dtype=np.float32)

    B, T, _ = x.shape
    slices_per_batch = n_cores // B

    # fold rmsnorm gains into weights (scale indexes the contracted dim)
    qw_f = q_w * (1.0 + pa)[None, :, None]
    kw_f = kv_w[0, 0] * (1.0 + pa)[:, None]
    vw_f = kv_w[1, 0] * (1.0 + pa)[:, None]
    w0 = gating_w[0] * (1.0 + pf)[:, None]
    w1 = gating_w[1] * (1.0 + pf)[:, None]
    wl = linear_w

    bf = lambda a: np.ascontiguousarray(a).astype(NPBF16)

    def hilo(arr):
        hi = np.clip(arr, -240.0, 240.0).astype(NPFP8)
        lo = (arr - hi.astype(np.float32)).astype(NPFP8)
        return hi, lo

    # attention weight packs (bf16, shared by all cores)
    qw_t = bf(qw_f.reshape(NH, DT, P, HC, P).transpose(0, 3, 2, 1, 4)
              .reshape(NH * HC, P, DT, P))
    kw_t = bf(kw_f.reshape(DT, P, HC, P).transpose(2, 1, 0, 3))
    vw_t = bf(vw_f.reshape(DT, P, HD).transpose(1, 0, 2))
    AVW = attn_vec_w.reshape(NH * HC * P, D)
    avw_t = bf(AVW.reshape(NH * HC, P, DT, P).transpose(2, 1, 0, 3))

    # MLP fp8 hi/lo packs
    # w01[g, k, di, ft, fi] = w_g[k*128+di, ft*128+fi] * s
    w01 = np.stack([w0 * wscales["s_w0"], w1 * wscales["s_w1"]], axis=0)
    w01 = w01.reshape(2, DT, P, FT, P)
    w01_hi, w01_lo = hilo(w01)
    # -> [ft, di, k, g, hl, fi]
    w01_hl = np.stack([w01_hi, w01_lo], axis=0)  # [hl, g, k, di, ft, fi]
    w01_t = np.ascontiguousarray(w01_hl.transpose(4, 3, 2, 1, 0, 5))
    # wl[p, jb, j4, fi, db, dc, di]
    wls = (wl * wscales["s_wl"]).reshape(NPORT, JPT // 4, 4, P, NDB, DBW, P)
    wl_hi, wl_lo = hilo(wls)
    wl_hl = np.stack([wl_hi, wl_lo], axis=0)  # [hl, p, jb, j4, fi, db, dc, di]
    # -> [p, db, jb, fi, j4, hl, dc, di]
    wl_t = np.ascontiguousarray(
        wl_hl.transpose(1, 5, 2, 4, 3, 0, 6, 7)
        .reshape(NPORT * NDB * (JPT // 4), P, 4, 2, DBW, P))

    # rope tables
    half = HD // 2
    ts_ = (10000.0 ** ((2.0 / HD) * np.arange(half, dtype=np.float32))
           ).astype(np.float32)
    rad = positions.astype(np.float32)[:, :, None] / ts_[None, None, :]
    cosT = np.cos(rad).transpose(0, 2, 1).astype(np.float32)  # [B, half, T]
    sinT = np.sin(rad).transpose(0, 2, 1).astype(np.float32)
    qscale = np.float32(HD) ** np.float32(-0.5)

    shared = dict(qw_t=qw_t, kw_t=kw_t, vw_t=vw_t, avw_t=avw_t,
                  w01_t=w01_t, wl_t=wl_t)

    in_maps = []
    for c in range(n_cores):
        b = c // slices_per_batch
        q0 = (c % slices_per_batch) * QT
        xt = x[b].T  # [D, T]
        xT_bf = bf(xt.reshape(DT, P, T).transpose(1, 0, 2))
        xTq_f32 = np.ascontiguousarray(
            xt[:, q0:q0 + QT].reshape(DT, P, QT).transpose(1, 0, 2)
        ).astype(np.float32)
        m = attn_mask[b, 0, q0:q0 + QT, :]  # [QT, S] bool
        maskT = bf(m.T.reshape(ST, P, QT).transpose(1, 0, 2).astype(np.float32))
        im = dict(
            xT_bf=xT_bf,
            xTq_f32=xTq_f32,
            xTq_bf=xT_bf[:, :, q0:q0 + QT].copy(),
            cosk=np.ascontiguousarray(cosT[b]),
            sink=np.ascontiguousarray(sinT[b]),
            cosq=np.ascontiguousarray(cosT[b][:, q0:q0 + QT] * qscale),
            sinq=np.ascontiguousarray(sinT[b][:, q0:q0 + QT] * qscale),
            maskT=maskT,
            **shared,
        )
        in_maps.append(im)
    return in_maps


def unpack_outputs(results, inputs, cfg=FULL_CFG, n_cores=8):
    D, S, QT, F, HD = _cfg_dims(cfg)
    x = np.asarray(inputs["x"])
    B, T, _ = x.shape
    slices_per_batch = n_cores // B
    out = np.empty((B, T, D), dtype=np.float32)
    for c in range(n_cores):
        b = c // slices_per_batch
        q0 = (c % slices_per_batch) * QT
        o = np.asarray(results[c]["outT"])  # [P, DT, QT]
        out[b, q0:q0 + QT, :] = o.transpose(2, 1, 0).reshape(QT, D)
    return out


_CACHE = {}
_CACHE_LOCK = threading.Lock()


def _get_nc(wscales, cfg_key="full"):
    with _CACHE_LOCK:
        if cfg_key not in _CACHE:
            _CACHE[cfg_key] = build_nc(wscales, FULL_CFG)
        return _CACHE[cfg_key]


def run(inputs, trace=False, **kwargs):
    from concourse.bass_utils import run_bass_kernel_spmd
    wscales = weight_scales(inputs)
    nc = _get_nc(wscales)
    in_maps = pack_inputs(inputs, wscales)
    res = run_bass_kernel_spmd(nc, in_maps, list(range(8)), trace=trace, **kwargs)
    out = unpack_outputs(res.results, inputs)
    return out, res


def kernel(**inputs):
    out, _ = run(inputs, trace=False)
    return out


# revision 23
# speedup vs baseline: 1.6848x; 1.6848x over previous
"""Trainium2 Bass kernel for a dense transformer block (GQA attention + GeGLU MLP).

Sharding: 8 cores = 2 batches x 4 token-slices of 512 tokens. Each core runs the
full block for its 512 query tokens; the (small) KV projection over the whole
batch is recomputed per core, so there is no cross-core communication.

Attention runs in bf16. The MLP matmuls run in fp8(e4m3) DoubleRow mode with
same-scale hi/lo decompositions to keep near-bf16 accuracy:
  - gate/up: weights split w = w_hi + w_lo; activations h = h_hi + h_lo.
    Per k-tile one DR instruction computes (w_hi + w_lo)@h_hi, and one DR
    instruction per k-tile PAIR adds w_hi@h_lo for both tiles (w_lo@h_lo is
    dropped: ~0.1% of the product). 3 instructions per 2 k-tiles vs 4 for bf16.
  - linear: weights hi/lo, activations single fp8 stored twice so one DR
    instruction per f-tile computes (wl_hi + wl_lo)@acts.

Layout convention on device: activations live transposed, [feature, token],
so every matmul contracts over the partition dim with natural operands.
"""

import sys
import threading

import numpy as np

sys.path.insert(0, "/opt/trn_rl_repo")

import ml_dtypes

import concourse.bass as bass
import concourse.tile as tile
from concourse import bacc
from concourse import mybir
from concourse.masks import make_identity

P = 128
F32 = mybir.dt.float32
BF16 = mybir.dt.bfloat16
FP8 = mybir.dt.float8e4
NPBF16 = ml_dtypes.bfloat16
NPFP8 = ml_dtypes.float8_e4m3
DR = mybir.MatmulPerfMode.DoubleRow

S_H2 = 32.0     # fp8 scale of the normalized mid residual (|h2|max ~ 5.5)
S_ACTS = 8.0    # fp8 scale of the geglu activations (|acts|max ~ 16)

FULL_CFG = dict(
    DT=16,    # d tiles (D = DT*128)
    ST=16,    # s tiles (S = ST*128) keys per batch
    NH=8,     # query heads
    HC=2,     # head-dim chunks (head dim = HC*128)
    QC=4,     # query-token chunks (QT = QC*128)
    FT=128,   # f tiles (F = FT*128)
    JPT=32,   # f tiles per portion
    NDB=4,    # d batches in linear sweep
    DBW=4,    # d chunks per linear batch
    EPS=1e-6,
)


def _cfg_dims(cfg):
    D = cfg["DT"] * P
    S = cfg["ST"] * P
    QT = cfg["QC"] * P
    F = cfg["FT"] * P
    HD = cfg["HC"] * P
    return D, S, QT, F, HD


def build_nc(cfg=FULL_CFG):
    DT, ST, NH, HC = cfg["DT"], cfg["ST"], cfg["NH"], cfg["HC"]
    QC, FT, JPT, NDB, DBW = cfg["QC"], cfg["FT"], cfg["JPT"], cfg["NDB"], cfg["DBW"]
    D, S, QT, F, HD = _cfg_dims(cfg)
    NPORT = FT // JPT
    assert NDB * DBW == DT
    EPS = cfg["EPS"]
    SC = S // 512          # 512-token chunks of the full batch
    CW = 512               # chunk width

    nc = bacc.Bacc(None)

    # ---- per-core inputs (host-prepared) ----
    xT_d = nc.dram_tensor("xT_bf", [P, DT, S], BF16, kind="ExternalInput")
    xTq_d = nc.dram_tensor("xTq_f32", [P, DT, QT], F32, kind="ExternalInput")
    xTqb_d = nc.dram_tensor("xTq_bf", [P, DT, QT], BF16, kind="ExternalInput")
    cosk_d = nc.dram_tensor("cosk", [P, S], F32, kind="ExternalInput")
    sink_d = nc.dram_tensor("sink", [P, S], F32, kind="ExternalInput")
    cosq_d = nc.dram_tensor("cosq", [P, QT], F32, kind="ExternalInput")
    sinq_d = nc.dram_tensor("sinq", [P, QT], F32, kind="ExternalInput")
    mask_d = nc.dram_tensor("maskT", [P, ST, QT], BF16, kind="ExternalInput")
    qw_d = nc.dram_tensor("qw_t", [NH * HC, P, DT, P], BF16, kind="ExternalInput")
    kw_d = nc.dram_tensor("kw_t", [HC, P, DT, P], BF16, kind="ExternalInput")
    vw_d = nc.dram_tensor("vw_t", [P, DT, HD], BF16, kind="ExternalInput")
    avw_d = nc.dram_tensor("avw_t", [DT, P, NH * HC, P], BF16, kind="ExternalInput")
    w01_d = nc.dram_tensor("w01_t", [FT, P, DT, 2, P], BF16,
                           kind="ExternalInput")
    # fp8 down-proj weights, per-output-channel scales, DoubleRow k-pair planes
    wl_d = nc.dram_tensor("wl_t", [NPORT * NDB * (JPT // 4), P, 2, DBW, 2, P],
                          FP8, kind="ExternalInput")
    dq_d = nc.dram_tensor("dq_t", [P, DT], F32, kind="ExternalInput")
    out_d = nc.dram_tensor("outT", [P, DT, QT], F32, kind="ExternalOutput")

    MULT = mybir.AluOpType.mult
    ADD = mybir.AluOpType.add
    SUB = mybir.AluOpType.subtract
    AF = mybir.ActivationFunctionType

    with tile.TileContext(nc) as tc:
        with (
            tc.tile_pool(name="glob", bufs=1) as glob,
            tc.tile_pool(name="misc", bufs=2) as misc,
            tc.tile_pool(name="dramp", bufs=1, space="DRAM") as dramp,
        ):
            ident = glob.tile([P, P], BF16, tag="ident")
            make_identity(nc, ident)
            id32 = glob.tile([P, P], F32, tag="id32")
            make_identity(nc, id32)
            ones_col = glob.tile([P, 1], BF16, tag="ones_col")
            nc.vector.memset(ones_col, 1.0)
            eps_b = glob.tile([P, 1], F32, tag="eps_b")
            nc.vector.memset(eps_b, EPS)
            zero_b = glob.tile([P, 1], F32, tag="zero_b")
            nc.vector.memset(zero_b, 0.0)
            dq_sb = glob.tile([P, DT], F32, tag="dq_sb")
            nc.sync.dma_start(dq_sb[:], dq_d[:])

            # PE warmup: keep TensorE busy while input DMAs land so the
            # HAM clock gate opens before real matmuls arrive.
            with tc.tile_pool(name="warm", bufs=1, space="PSUM") as warmp:
                wps = warmp.tile([P, P], F32, tag="warm")
                for wi in range(40):
                    nc.tensor.matmul(wps, ident[:], ident[:],
                                     start=(wi == 0), stop=(wi == 39))

            # cross-phase tensors
            kT = glob.tile([P, HC, S], BF16, tag="kT")
            v_sb = glob.tile([P, ST, HD + 1], BF16, tag="v_sb")
            qT = glob.tile([P, NH * HC, QT], BF16, tag="qT")
            xTq = glob.tile([P, DT, QT], F32, tag="bigf32")   # later reused by h2T
            x_newT = glob.tile([P, DT, QT], F32, tag="x_newT")

            def rstd_chunk(sq, ps_pool, ps_tag, nat, col0, nq, sc, bias):
                """sq: [P, DT, nq*128] bf16; writes rstd into nat[:, col0:col0+nq]."""
                for qq in range(nq):
                    var_ps = ps_pool.tile([P, 1], F32, tag=ps_tag, bufs=2, name="var_ps")
                    for dt_i in range(DT):
                        nc.tensor.matmul(
                            var_ps, sq[:, dt_i, qq * P:(qq + 1) * P], ones_col,
                            start=(dt_i == 0), stop=(dt_i == DT - 1))
                    std = misc.tile([P, 1], F32, tag="std")
                    nc.scalar.activation(std, var_ps, AF.Sqrt,
                                         bias=bias[:], scale=sc)
                    nc.vector.reciprocal(nat[:, col0 + qq:col0 + qq + 1], std)

            def rstd_broadcast(nat, n_tiles, ps_pool, ps_tag2, bc_pool, bc_tag):
                """nat: [P, n_tiles] f32 -> [P, n_tiles*128] broadcast tile."""
                tps = ps_pool.tile([P, P], F32, tag=ps_tag2, bufs=2, name="tps")
                nc.tensor.transpose(tps[:n_tiles, :], nat[:], id32)
                row_sb = misc.tile([n_tiles, P], F32, tag=f"row{n_tiles}", bufs=1)
                nc.vector.tensor_copy(row_sb[:], tps[:n_tiles, :])
                scr = dramp.tile([n_tiles, P], F32, tag=f"scr{bc_tag}")
                nc.sync.dma_start(scr[:], row_sb[:])
                bc = bc_pool.tile([P, n_tiles * P], F32, tag=bc_tag, bufs=1, name="bc")
                src_ap = bass.AP(tensor=scr.tensor, offset=scr.offset,
                                 ap=[[0, P], [1, n_tiles * P]])
                nc.gpsimd.dma_start(bc[:], src_ap)
                return bc

            # =========== Phase P: norms, projections, RoPE ===========
            with (
                tc.tile_pool(name="pp", bufs=1) as pp,
                tc.tile_pool(name="ppr", bufs=2) as ppr,
                tc.tile_pool(name="wtile", bufs=2) as wtile,
                tc.tile_pool(name="psP", bufs=1, space="PSUM") as psP,
            ):
                # ---- N1q: rmsnorm of this core's q slice ----
                xTqb = pp.tile([P, DT, QT], BF16, tag="big16", bufs=2,
                               name="xTqb")
                sqq = pp.tile([P, DT, QT], BF16, tag="sq", bufs=1, name="sqq")
                natq = misc.tile([P, QC], F32, tag="natq")
                for qq in range(QC):
                    qsl = slice(qq * P, (qq + 1) * P)
                    nc.sync.dma_start(xTqb[:, :, qsl], xTqb_d[:, :, qsl])
                    nc.vector.tensor_tensor(sqq[:, :, qsl], xTqb[:, :, qsl],
                                            xTqb[:, :, qsl], MULT)
                    rstd_chunk(sqq[:, :, qsl], psP, "var", natq, qq, 1,
                               1.0 / D, eps_b)
                nc.sync.dma_start(xTq[:], xTq_d[:])
                bcq = rstd_broadcast(natq, QC, psP, "tps", ppr, "bcq")
                normTq = pp.tile([P, DT, QT], BF16, tag="nq16", bufs=1,
                                 name="normTq")
                for dt_i in range(DT):
                    nc.vector.tensor_tensor(normTq[:, dt_i, :], xTq[:, dt_i, :],
                                            bcq[:], MULT)

                cosq = pp.tile([P, QT], F32, tag="cosq")
                nc.sync.dma_start(cosq[:], cosq_d[:])
                sinq = pp.tile([P, QT], F32, tag="sinq")
                nc.sync.dma_start(sinq[:], sinq_d[:])

                def rope_pair(x1_ps, x2_ps, cos_sl, sin_sl, out1, out2, w):
                    t1 = ppr.tile([P, w], F32, tag="ropeA", bufs=1, name="t1")
                    t2 = ppr.tile([P, w], F32, tag="ropeB", bufs=1, name="t2")
                    nc.vector.tensor_tensor(t1[:], x1_ps[:], cos_sl, MULT)
                    nc.vector.tensor_tensor(t2[:], x2_ps[:], sin_sl, MULT)
                    nc.vector.tensor_tensor(out1, t1[:], t2[:], SUB)
                    t3 = ppr.tile([P, w], F32, tag="ropeA", bufs=1, name="t3")
                    t4 = ppr.tile([P, w], F32, tag="ropeB", bufs=1, name="t4")
                    nc.vector.tensor_tensor(t3[:], x2_ps[:], cos_sl, MULT)
                    nc.vector.tensor_tensor(t4[:], x1_ps[:], sin_sl, MULT)
                    nc.vector.tensor_tensor(out2, t3[:], t4[:], ADD)

                def q_head(n):
                    qps = []
                    for hc in range(HC):
                        qw_sb = wtile.tile([P, DT, P], BF16, tag="wt",
                                           name="qw_sb")
                        nc.sync.dma_start(qw_sb[:], qw_d[n * HC + hc])
                        qp = psP.tile([P, QT], F32, tag="pj", bufs=4, name="qp")
                        for dt_i in range(DT):
                            nc.tensor.matmul(qp, qw_sb[:, dt_i, :],
                                             normTq[:, dt_i, :],
                                             start=(dt_i == 0),
                                             stop=(dt_i == DT - 1))
                        qps.append(qp)
                    rope_pair(qps[0], qps[1], cosq[:], sinq[:],
                              qT[:, n * HC, :], qT[:, n * HC + 1, :], QT)

                q_head(0)

                # ---- full-batch stats pass: rstd for every s-chunk ----
                bc_list = []
                for c in range(SC):
                    csl = slice(c * CW, (c + 1) * CW)
                    xTa = pp.tile([P, DT, CW], BF16, tag="big16", bufs=2,
                                  name="xTa")
                    nc.sync.dma_start(xTa[:], xT_d[:, :, csl])
                    sqc = pp.tile([P, DT, CW], BF16, tag="sq", bufs=1,
                                  name="sqc")
                    nc.vector.tensor_tensor(sqc[:], xTa[:], xTa[:], MULT)
                    natc = misc.tile([P, CW // P], F32, tag="natc")
                    rstd_chunk(sqc, psP, "var", natc, 0, CW // P, 1.0 / D,
                               eps_b)
                    bcc = rstd_broadcast(natc, CW // P, psP, "var", ppr,
                                         f"bcc{c}")
                    bc_list.append(bcc)

                for n in range(1, NH):
                    q_head(n)
                nc.sync.dma_start(xTq[:], xTq_d[:])

                # ---- norm + K/V projections (re-load x chunks) ----
                kw_sbs = []
                for hc in range(HC):
                    kw_sb = wtile.tile([P, DT, P], BF16, tag="wt",
                                       name="kw_sb")
                    nc.sync.dma_start(kw_sb[:], kw_d[hc])
                    kw_sbs.append(kw_sb)
                vw_sb = wtile.tile([P, DT, HD], BF16, tag="vwt", bufs=1,
                                   name="vw_sb")
                nc.sync.dma_start(vw_sb[:], vw_d[:])
                nc.vector.memset(v_sb[:], 1.0)

                for c in range(SC):
                    csl = slice(c * CW, (c + 1) * CW)
                    xTc = pp.tile([P, DT, CW], BF16, tag="big16", bufs=2,
                                  name="xTc")
                    nc.sync.dma_start(xTc[:], xT_d[:, :, csl])
                    for dt_i in range(DT):
                        nc.vector.tensor_tensor(xTc[:, dt_i, :], xTc[:, dt_i, :],
                                                bc_list[c][:], MULT)
                    # K^T for this chunk
                    cos_c = ppr.tile([P, CW], F32, tag="cos_c", bufs=1, name="cos_c")
                    nc.sync.dma_start(cos_c[:], cosk_d[:, csl])
                    sin_c = ppr.tile([P, CW], F32, tag="sin_c", bufs=1, name="sin_c")
                    nc.sync.dma_start(sin_c[:], sink_d[:, csl])
                    kps = []
                    for hc in range(HC):
                        kp = psP.tile([P, CW], F32, tag="pj", bufs=4, name="kp")
                        for dt_i in range(DT):
                            nc.tensor.matmul(kp, kw_sbs[hc][:, dt_i, :],
                                             xTc[:, dt_i, :],
                                             start=(dt_i == 0),
                                             stop=(dt_i == DT - 1))
                        kps.append(kp)
                    rope_pair(kps[0], kps[1], cos_c[:], sin_c[:],
                              kT[:, 0, csl], kT[:, 1, csl], CW)
                    # V for this chunk
                    for ss in range(CW // P):
                        st_i = c * (CW // P) + ss
                        vp = psP.tile([P, HD], F32, tag="pj", bufs=4, name="vp")
                        for dt_i in range(DT):
                            nc.tensor.matmul(
                                vp, xTc[:, dt_i, ss * P:(ss + 1) * P],
                                vw_sb[:, dt_i, :],
                                start=(dt_i == 0), stop=(dt_i == DT - 1))
                        nc.scalar.copy(v_sb[:, st_i, :HD], vp[:])

            # =========== Phase A: attention, out-proj, N2 ===========
            with (
                tc.tile_pool(name="pa", bufs=1) as pa,
                tc.tile_pool(name="expp", bufs=2) as expp,
                tc.tile_pool(name="avwp", bufs=3) as avwp,
            ):
                mask_sb = pa.tile([P, ST, QT], BF16, tag="m16", name="mask_sb")
                nc.sync.dma_start(mask_sb[:], mask_d[:])
                enc_sb = pa.tile([P, QC, NH, HD], BF16, tag="enc_sb")
                encT = pa.tile([P, NH * HC, QT], BF16, tag="encT")
                exp_tiles = [None] * NH

                with tc.tile_pool(name="psA", bufs=1, space="PSUM") as psA:

                    def emit_logits(n):
                        expT = expp.tile([P, ST, QT], BF16, tag="expT",
                                         name="expT")
                        exp_tiles[n] = expT
                        for st_i in range(ST):
                            lp = psA.tile([P, QT], F32, tag="lps", bufs=4,
                                          name="lp")
                            for hc in range(HC):
                                nc.tensor.matmul(
                                    lp, kT[:, hc, st_i * P:(st_i + 1) * P],
                                    qT[:, n * HC + hc, :],
                                    start=(hc == 0), stop=(hc == HC - 1))
                            nc.scalar.activation(expT[:, st_i, :], lp, AF.Exp,
                                                 bias=zero_b[:])
                            nc.vector.tensor_tensor(expT[:, st_i, :],
                                                    expT[:, st_i, :],
                                                    mask_sb[:, st_i, :], MULT)

                    def emit_av(n):
                        expT = exp_tiles[n]
                        for qc in range(QC):
                            ap_ = psA.tile([P, HD + 1], F32, tag="avps",
                                           bufs=2, name="ap_")
                            for st_i in range(ST):
                                nc.tensor.matmul(
                                    ap_, expT[:, st_i, qc * P:(qc + 1) * P],
                                    v_sb[:, st_i, :],
                                    start=(st_i == 0), stop=(st_i == ST - 1))
                            ssum = misc.tile([P, 1], F32, tag="ssum")
                            nc.vector.tensor_copy(ssum[:], ap_[:, HD:HD + 1])
                            rec = misc.tile([P, 1], F32, tag="rec")
                            nc.vector.reciprocal(rec[:], ssum[:])
                            nc.vector.tensor_scalar_mul(enc_sb[:, qc, n, :],
                                                        ap_[:, :HD], rec[:])

                    for n in range(NH):
                        emit_logits(n)
                        if n > 0:
                            emit_av(n - 1)
                    emit_av(NH - 1)

                    # transpose enc -> encT [nh*hd, qt]
                    for qc in range(QC):
                        for nt in range(NH * HC):
                            n, hs = nt // HC, nt % HC
                            tp = psA.tile([P, P], BF16, tag="lps", bufs=4,
                                          name="tp")
                            nc.tensor.transpose(
                                tp, enc_sb[:, qc, n, hs * P:(hs + 1) * P],
                                ident)
                            nc.vector.tensor_copy(
                                encT[:, nt, qc * P:(qc + 1) * P], tp[:])

                with tc.tile_pool(name="psA2", bufs=1, space="PSUM") as psA2:
                    # out-projection (transposed) + residual; N2 variance
                    # matmuls accumulate inside the loop.
                    sq2 = pa.tile([P, DT, QT], BF16, tag="m16", name="sq2")
                    var2 = [
                        psA2.tile([P, 1], F32, tag=f"n2v{i}", bufs=1,
                                  name=f"n2v{i}")
                        for i in range(QC)
                    ]
                    for dc in range(DT):
                        avw_sb = avwp.tile([P, NH * HC, P], BF16, tag="avw",
                                           name="avw_sb")
                        nc.scalar.dma_start(avw_sb[:], avw_d[dc])
                        op_ = psA2.tile([P, QT], F32, tag="ops", bufs=2,
                                        name="op_")
                        for nt in range(NH * HC):
                            nc.tensor.matmul(op_, avw_sb[:, nt, :],
                                             encT[:, nt, :],
                                             start=(nt == 0),
                                             stop=(nt == NH * HC - 1))
                        nc.vector.tensor_tensor(x_newT[:, dc, :], op_[:],
                                                xTq[:, dc, :], ADD)
                        nc.vector.tensor_tensor(sq2[:, dc, :],
                                                x_newT[:, dc, :],
                                                x_newT[:, dc, :], MULT)
                        for qq in range(QC):
                            nc.tensor.matmul(
                                var2[qq], sq2[:, dc, qq * P:(qq + 1) * P],
                                ones_col, start=(dc == 0), stop=(dc == DT - 1))

                    # ---- N2 tail ----
                    nat2 = misc.tile([P, QC], F32, tag="nat2")
                    for qq in range(QC):
                        std2 = misc.tile([P, 1], F32, tag="std")
                        nc.scalar.activation(std2, var2[qq], AF.Sqrt,
                                             bias=eps_b[:], scale=1.0 / D)
                        nc.vector.reciprocal(nat2[:, qq:qq + 1], std2)
                    bc2 = rstd_broadcast(nat2, QC, psA2, "ops", pa, "bc2")
                    h2T = glob.tile([P, DT, QT], BF16, tag="bigf32", name="h2T")
                    for dt_i in range(DT):
                        nc.vector.tensor_tensor(h2T[:, dt_i, :],
                                                x_newT[:, dt_i, :], bc2[:],
                                                MULT)

            # ====== Phase M: GeGLU MLP (bf16 gate/up, fp8 DR linear) ======
            with (
                tc.tile_pool(name="actsp", bufs=1) as actsp,
                tc.tile_pool(name="w01p", bufs=3) as w01p,
                tc.tile_pool(name="wlp", bufs=3) as wlp,
                tc.tile_pool(name="gelp", bufs=3) as gelp,
                tc.tile_pool(name="upscp", bufs=3) as upscp,
                tc.tile_pool(name="linsp", bufs=3) as linsp,
                tc.tile_pool(name="psM", bufs=1, space="PSUM") as psM,
            ):
                for p in range(NPORT):
                    actsT = actsp.tile([P, JPT, QT], FP8, tag="acts",
                                       name="actsT")
                    for j in range(JPT):
                        ft = p * JPT + j
                        w01_sb = w01p.tile([P, DT, 2, P], BF16, tag="w01",
                                           name="w01_sb")
                        nc.sync.dma_start(w01_sb[:], w01_d[ft])
                        gp = psM.tile([P, QT], F32, tag="gps", bufs=2, name="gp")
                        for dt_i in range(DT):
                            nc.tensor.matmul(gp, w01_sb[:, dt_i, 0, :],
                                             h2T[:, dt_i, :],
                                             start=(dt_i == 0),
                                             stop=(dt_i == DT - 1))
                        gel = gelp.tile([P, QT], BF16, tag="gel", name="gel")
                        nc.scalar.activation(gel[:], gp, AF.Gelu_apprx_tanh,
                                             bias=zero_b[:])
                        up = psM.tile([P, QT], F32, tag="ups", bufs=2, name="up")
                        for dt_i in range(DT):
                            nc.tensor.matmul(up, w01_sb[:, dt_i, 1, :],
                                             h2T[:, dt_i, :],
                                             start=(dt_i == 0),
                                             stop=(dt_i == DT - 1))
                        # acts in fp8, scaled by S_ACTS (folded via up)
                        up_sc = upscp.tile([P, QT], BF16, tag="upsc",
                                           name="up_sc")
                        nc.scalar.activation(up_sc[:], up, AF.Copy,
                                             scale=S_ACTS)
                        nc.vector.tensor_tensor(actsT[:, j, :], up_sc[:],
                                                gel[:], MULT)
                    for db in range(NDB):
                        lps = [
                            psM.tile([P, QT], F32, tag=f"lin{i}", bufs=1,
                                     name=f"lin{i}")
                            for i in range(DBW)
                        ]
                        for jb in range(JPT // 4):
                            wl_sb = wlp.tile([P, 2, DBW, 2, P], FP8, tag="wl",
                                             name="wl_sb")
                            nc.scalar.dma_start(
                                wl_sb[:], wl_d[(p * NDB + db) * (JPT // 4) + jb])
                            for jp2 in range(2):
                                jpair = jb * 2 + jp2
                                j0 = jb * 4 + jp2 * 2
                                for dc4 in range(DBW):
                                    nc.tensor.matmul(
                                        lps[dc4],
                                        wl_sb[:, jp2, dc4, :, :],
                                        actsT[:, j0:j0 + 2, :],
                                        start=(jpair == 0),
                                        stop=(jpair == JPT // 2 - 1),
                                        perf_mode=DR)
                        for dc4 in range(DBW):
                            d_i = db * DBW + dc4
                            lin_sc = linsp.tile([P, QT], F32, tag="linsc",
                                                name="lin_sc")
                            nc.scalar.activation(lin_sc[:], lps[dc4], AF.Copy,
                                                 scale=dq_sb[:, d_i:d_i + 1])
                            nc.vector.tensor_tensor(x_newT[:, d_i, :],
                                                    lin_sc[:],
                                                    x_newT[:, d_i, :], ADD)
                        if p == NPORT - 1:
                            nc.sync.dma_start(
                                out_d[:, db * DBW:(db + 1) * DBW, :],
                                x_newT[:, db * DBW:(db + 1) * DBW, :])

    nc.compile()
    return nc


# ---------------------------------------------------------------------------
# Host-side packing
# ---------------------------------------------------------------------------

def pack_inputs(inputs, cfg=FULL_CFG, n_cores=8):
    D, S, QT, F, HD = _cfg_dims(cfg)
    DT, ST, NH, HC = cfg["DT"], cfg["ST"], cfg["NH"], cfg["HC"]
    QC, FT, JPT, NDB, DBW = cfg["QC"], cfg["FT"], cfg["JPT"], cfg["NDB"], cfg["DBW"]
    NPORT = FT // JPT

    x = np.asarray(inputs["x"], dtype=np.float32)            # [B, T, D]
    positions = np.asarray(inputs["positions"])              # [B, T]
    attn_mask = np.asarray(inputs["attn_mask"])              # [B, 1, T, T]
    q_w = np.asarray(inputs["q_w"], dtype=np.float32)        # [NH, D, HD]
    kv_w = np.asarray(inputs["kv_w"], dtype=np.float32)      # [2, 1, D, HD]
    attn_vec_w = np.asarray(inputs["attn_vec_w"], dtype=np.float32)  # [NH, HD, D]
    gating_w = np.asarray(inputs["gating_w"], dtype=np.float32)      # [2, D, F]
    linear_w = np.asarray(inputs["linear_w"], dtype=np.float32)      # [F, D]
    pa = np.asarray(inputs["pre_attn_scale"], dtype=np.float32)
    pf = np.asarray(inputs["pre_ffw_scale"], dtype=np.float32)

    B, T, _ = x.shape
    slices_per_batch = n_cores // B

    # fold rmsnorm gains into weights (scale indexes the contracted dim)
    qw_f = q_w * (1.0 + pa)[None, :, None]
    kw_f = kv_w[0, 0] * (1.0 + pa)[:, None]
    vw_f = kv_w[1, 0] * (1.0 + pa)[:, None]
    w0 = gating_w[0] * (1.0 + pf)[:, None]
    w1 = gating_w[1] * (1.0 + pf)[:, None]
    wl = linear_w

    bf = lambda a: np.ascontiguousarray(a).astype(NPBF16)

    # attention + gate/up weight packs (bf16, shared by all cores)
    qw_t = bf(qw_f.reshape(NH, DT, P, HC, P).transpose(0, 3, 2, 1, 4)
              .reshape(NH * HC, P, DT, P))
    kw_t = bf(kw_f.reshape(DT, P, HC, P).transpose(2, 1, 0, 3))
    vw_t = bf(vw_f.reshape(DT, P, HD).transpose(1, 0, 2))
    AVW = attn_vec_w.reshape(NH * HC * P, D)
    avw_t = bf(AVW.reshape(NH * HC, P, DT, P).transpose(2, 1, 0, 3))
    w01 = np.stack([w0.reshape(DT, P, FT, P), w1.reshape(DT, P, FT, P)],
                   axis=2)                       # [DT, P(di), 2, FT, P(fi)]
    w01_t = bf(w01.transpose(3, 1, 0, 2, 4))     # [FT, P, DT, 2, P]

    # down-proj in fp8 with per-output-channel scales
    s_wl_col = 240.0 / np.abs(wl).max(axis=0)    # [D]
    wls = wl * s_wl_col[None, :]
    # wl_t[(p*NDB+db)*(JPT//4)+jb, fi, jp, dc, jj, di]
    #   = wls[(p*JPT+jb*4+jp*2+jj)*128+fi, (db*DBW+dc)*128+di]
    wl_t = (wls.reshape(NPORT, JPT // 4, 2, 2, P, NDB, DBW, P)
            .transpose(0, 5, 1, 4, 2, 6, 3, 7)   # [p, db, jb, fi, jp, dc, jj, di]
            .reshape(NPORT * NDB * (JPT // 4), P, 2, DBW, 2, P))
    wl_t = np.clip(wl_t, -240.0, 240.0).astype(NPFP8)
    # dq_t[p_row, d_i] = 1 / (S_ACTS * s_wl_col[d_i*128 + p_row])
    S_ACTS = 8.0
    dq_t = np.ascontiguousarray(
        (1.0 / (S_ACTS * s_wl_col)).reshape(DT, P).T).astype(np.float32)

    # rope tables
    half = HD // 2
    ts_ = (10000.0 ** ((2.0 / HD) * np.arange(half, dtype=np.float32))
           ).astype(np.float32)
    rad = positions.astype(np.float32)[:, :, None] / ts_[None, None, :]
    cosT = np.cos(rad).transpose(0, 2, 1).astype(np.float32)  # [B, half, T]
    sinT = np.sin(rad).transpose(0, 2, 1).astype(np.float32)
    qscale = np.float32(HD) ** np.float32(-0.5)

    shared = dict(qw_t=qw_t, kw_t=kw_t, vw_t=vw_t, avw_t=avw_t,
                  w01_t=w01_t, wl_t=wl_t, dq_t=dq_t)

    in_maps = []
    for c in range(n_cores):
        b = c // slices_per_batch
        q0 = (c % slices_per_batch) * QT
        xt = x[b].T  # [D, T]
        xT_bf = bf(xt.reshape(DT, P, T).transpose(1, 0, 2))
        xTq_f32 = np.ascontiguousarray(
            xt[:, q0:q0 + QT].reshape(DT, P, QT).transpose(1, 0, 2)
        ).astype(np.float32)
        m = attn_mask[b, 0, q0:q0 + QT, :]  # [QT, S] bool
        maskT = bf(m.T.reshape(ST, P, QT).transpose(1, 0, 2).astype(np.float32))
        im = dict(
            xT_bf=xT_bf,
            xTq_f32=xTq_f32,
            xTq_bf=xT_bf[:, :, q0:q0 + QT].copy(),
            cosk=np.ascontiguousarray(cosT[b]),
            sink=np.ascontiguousarray(sinT[b]),
            cosq=np.ascontiguousarray(cosT[b][:, q0:q0 + QT] * qscale),
            sinq=np.ascontiguousarray(sinT[b][:, q0:q0 + QT] * qscale),
            maskT=maskT,
            **shared,
        )
        in_maps.append(im)
    return in_maps


def unpack_outputs(results, inputs, cfg=FULL_CFG, n_cores=8):
    D, S, QT, F, HD = _cfg_dims(cfg)
    x = np.asarray(inputs["x"])
    B, T, _ = x.shape
    slices_per_batch = n_cores // B
    out = np.empty((B, T, D), dtype=np.float32)
    for c in range(n_cores):
        b = c // slices_per_batch
        q0 = (c % slices_per_batch) * QT
        o = np.asarray(results[c]["outT"])  # [P, DT, QT]
        out[b, q0:q0 + QT, :] = o.transpose(2, 1, 0).reshape(QT, D)
    return out


_CACHE = {}
_CACHE_LOCK = threading.Lock()


def _get_nc(cfg_key="full"):
    with _CACHE_LOCK:
        if cfg_key not in _CACHE:
            _CACHE[cfg_key] = build_nc(FULL_CFG)
        return _CACHE[cfg_key]


def run(inputs, trace=False, **kwargs):
    from concourse.bass_utils import run_bass_kernel_spmd
    nc = _get_nc()
    in_maps = pack_inputs(inputs)
    res = run_bass_kernel_spmd(nc, in_maps, list(range(8)), trace=trace, **kwargs)
    out = unpack_outputs(res.results, inputs)
    return out, res


def kernel(**inputs):
    out, _ = run(inputs, trace=False)
    return out
